# revision 6
# baseline (speedup 1.0000x reference)
"""Causal attention kernel for Trainium2 (8 NeuronCores).

Problem: B=2, H=16, S=2048, D=64 causal attention with a softmax whose
global-max subtraction cancels mathematically (softmax is shift-invariant),
so an unshifted softmax is numerically equivalent in f32.

Sharding: the 32 (b,h) heads are split 4-per-core across 8 cores
(head-parallel, no communication).

Per-core kernel (per head):
  - Load Q,K transposed ([D=64, S] layout, d on partitions) via DMA transpose.
  - Load V natural [S, D] as 16 chunks of [128, 64] with a ones column
    appended ([128, 65]) so the PV matmul accumulates softmax row-sums for
    free in output row 64.
  - For each q-block of 512 and each causal k-chunk of 128:
      S^T[k,q] = matmul(lhsT=Kt chunk, rhs=Qt block)   (float32r, full speed)
      e = exp(0.125 * S^T)  on ScalarE (PSUM -> SBUF)
      straddle blocks: multiply by staircase 0/1 mask (VectorE)
      PV psum[65, 512] += matmul(lhsT=Vplus chunk, rhs=e)
  - Epilogue: copy PV psum to SBUF, PE-transpose [65,128] blocks to
    [128,65], out = cols 0..63 * reciprocal(col 64), DMA out.
"""

import numpy as np

B, H, S, D = 2, 16, 2048, 64
N_CORES = 8
HPC = (B * H) // N_CORES  # heads per core = 4
QB = 512  # q-block width
KB = 128  # k-chunk width
NQB = S // QB  # 4
NKB = S // KB  # 16

_CACHED = {}


def _build_nc():
    import concourse.bacc as bacc
    import concourse.mybir as mybir
    from concourse.tile import TileContext
    from concourse.masks import make_identity

    f32 = mybir.dt.float32
    f32r = mybir.dt.float32r
    EXP = mybir.ActivationFunctionType.Exp

    nc = bacc.Bacc()
    # Q and K are pre-transposed on the host to [head, D, S] so the on-chip
    # [d, s] layout (contraction dim d on partitions) loads with contiguous DMA.
    Qd = nc.declare_dram_parameter("Qt", [HPC, D, S], f32, isOutput=False)
    Kd = nc.declare_dram_parameter("Kt", [HPC, D, S], f32, isOutput=False)
    Vd = nc.declare_dram_parameter("V", [HPC, S, D], f32, isOutput=False)
    Od = nc.declare_dram_parameter("out", [HPC, S, D], f32, isOutput=True)

    with TileContext(nc) as tc:
        with (
            tc.tile_pool(name="consts", bufs=1) as cpool,
            tc.tile_pool(name="qt", bufs=2) as qt_pool,
            tc.tile_pool(name="kt", bufs=2) as kt_pool,
            tc.tile_pool(name="vp", bufs=2) as v_pool,
            tc.tile_pool(name="e", bufs=4) as e_pool,
            tc.tile_pool(name="ot", bufs=2) as ot_pool,
            tc.tile_pool(name="oo", bufs=3) as oo_pool,
            tc.tile_pool(name="r", bufs=3) as r_pool,
            tc.tile_pool(name="ps", bufs=3, space="PSUM") as ps_pool,
            tc.tile_pool(name="po", bufs=2, space="PSUM") as po_pool,
            tc.tile_pool(name="pt", bufs=2, space="PSUM") as pt_pool,
        ):
            # constants: identity for PE transpose, staircase causal mask
            ident = cpool.tile([128, 128], f32)
            make_identity(nc, ident[:])
            # Mbig[i, jj] = 1.0 if (jj - i) >= 384 else 0.0
            # straddle chunk with offset off = k0-q0 uses Mbig[:, 384-off : 896-off]
            mbig = cpool.tile([128, 896], f32)
            nc.gpsimd.memset(mbig[:], 0.0)
            nc.gpsimd.affine_select(
                out=mbig[:],
                in_=mbig[:],
                compare_op=mybir.AluOpType.is_ge,
                fill=-1e30,
                base=-384,
                pattern=[[1, 896]],
                channel_multiplier=-1,
            )

            for h in range(HPC):
                # ---- load & transpose inputs for this head ----
                qt = qt_pool.tile([D, S], f32r, tag="qt")
                nc.sync.dma_start(out=qt[:], in_=Qd[h].bitcast(f32r))
                kt = kt_pool.tile([D, S], f32r, tag="kt")
                nc.sync.dma_start(out=kt[:], in_=Kd[h].bitcast(f32r))
                vp = v_pool.tile([128, NKB, KB // 2 + 1], f32r, tag="vp")  # [128,16,65]
                nc.sync.dma_start(
                    out=vp[:, :, 0:D],
                    in_=Vd[h].rearrange("(c p) d -> p c d", p=128).bitcast(f32r),
                )
                nc.gpsimd.memset(vp[:, :, D].bitcast(f32), 1.0)

                for qb in range(NQB):
                    q0 = qb * QB
                    nk = (q0 + QB) // KB  # causal: k-chunks 0..nk-1
                    po = po_pool.tile([D + 1, QB], f32, tag="po")

                    pending = []  # software pipeline: PV lags one block

                    def emit_pv(ki, e_tile, first, last):
                        nc.tensor.matmul(
                            po[:],
                            lhsT=vp[:, ki, :],
                            rhs=e_tile[:],
                            start=first,
                            stop=last,
                        )

                    for ki in range(nk):
                        k0 = ki * KB
                        ps = ps_pool.tile([KB, QB], f32, tag="ps")
                        nc.tensor.matmul(
                            ps[:],
                            lhsT=kt[:, k0 : k0 + KB],
                            rhs=qt[:, q0 : q0 + QB],
                            start=True,
                            stop=True,
                        )
                        if k0 >= q0:  # straddles the diagonal -> mask scores
                            off = k0 - q0
                            nc.vector.tensor_add(
                                ps[:], ps[:], mbig[:, 384 - off : 896 - off]
                            )
                        e = e_pool.tile([KB, QB], f32r, tag="e")
                        nc.scalar.activation(e[:], ps[:], EXP, scale=0.125)
                        pending.append((ki, e))
                        if len(pending) > 1:
                            pki, pe = pending.pop(0)
                            emit_pv(pki, pe, pki == 0, pki == nk - 1)
                    pki, pe = pending.pop(0)
                    emit_pv(pki, pe, pki == 0, pki == nk - 1)

                    # ---- epilogue: transpose + normalize + store ----
                    ot = ot_pool.tile([D + 1, QB], f32, tag="ot")
                    nc.vector.tensor_copy(ot[:], po[:])
                    for j in range(QB // 128):
                        pt = pt_pool.tile([128, D + 1], f32, tag="pt")
                        nc.tensor.transpose(
                            pt[:], ot[:, j * 128 : (j + 1) * 128], ident[: D + 1, : D + 1]
                        )
                        r = r_pool.tile([128, 1], f32, tag="r")
                        nc.vector.reciprocal(r[:], pt[:, D : D + 1])
                        oo = oo_pool.tile([128, D], f32, tag="oo")
                        nc.vector.tensor_scalar_mul(oo[:], pt[:, 0:D], r[:])
                        nc.sync.dma_start(
                            out=Od[h, q0 + j * 128 : q0 + (j + 1) * 128, :],
                            in_=oo[:],
                        )
    nc.finalize()
    return nc


def _get_nc():
    if "nc" not in _CACHED:
        _CACHED["nc"] = _build_nc()
    return _CACHED["nc"]


def kernel(Q, K, V, mask=None, **_ignored):
    from concourse.bass_utils import run_bass_kernel_spmd

    nc = _get_nc()
    Qr = np.ascontiguousarray(
        np.asarray(Q, dtype=np.float32).reshape(B * H, S, D).transpose(0, 2, 1)
    )
    Kr = np.ascontiguousarray(
        np.asarray(K, dtype=np.float32).reshape(B * H, S, D).transpose(0, 2, 1)
    )
    Vr = np.ascontiguousarray(np.asarray(V, dtype=np.float32).reshape(B * H, S, D))
    in_maps = [
        {
            "Qt": Qr[i * HPC : (i + 1) * HPC],
            "Kt": Kr[i * HPC : (i + 1) * HPC],
            "V": Vr[i * HPC : (i + 1) * HPC],
        }
        for i in range(N_CORES)
    ]
    res = run_bass_kernel_spmd(nc, in_maps, core_ids=list(range(N_CORES)))
    out = np.concatenate([res.results[i]["out"] for i in range(N_CORES)], axis=0)
    return out.reshape(B, H, S, D).astype(np.float32)


# revision 7
# speedup vs baseline: 1.2128x; 1.2128x over previous
"""Causal attention kernel for Trainium2 (8 NeuronCores).

Problem: B=2, H=16, S=2048, D=64 causal attention with a softmax whose
global-max subtraction cancels mathematically (softmax is shift-invariant),
so an unshifted softmax is numerically equivalent in f32.

Sharding: the 32 (b,h) heads are split 4-per-core across 8 cores
(head-parallel, no communication). Q and K are pre-transposed on the host to
[head, D, S] during shard prep so the on-chip [d, s] layout (contraction dim
d on partitions) loads with contiguous DMA.

Per-core kernel (per head, scores computed in S^T = [k, q] layout):
  - QK: S^T[k_chunk, q_block] = matmul(lhsT=Kt chunk [64,128],
    rhs=Qt block [64,512]) in float32r (FP22 multiply, full PE speed).
  - exp(0.125 * S^T) on ScalarE straight out of PSUM; full (sub-diagonal)
    chunk pairs share one [128,1024] activation to amortize ACT overhead;
    diagonal-straddling chunks exp only the causal suffix, the masked prefix
    of e is zeroed on GpSimd, and the [128,128] diagonal sub-block gets a
    -1e30 additive mask on VectorE before the exp.
  - PV: psum[65, 512] += matmul(lhsT=Vplus chunk [128,65], rhs=e) where
    Vplus has a ones column appended so row 64 accumulates softmax row-sums.
  - Epilogue: copy PV psum to SBUF, 4 PE-transposes into one [128, 4*65]
    PSUM tile, single reciprocal + broadcast multiply, one DMA per q-block.
"""

import numpy as np

B, H, S, D = 2, 16, 2048, 64
N_CORES = 8
HPC = (B * H) // N_CORES  # heads per core = 4
QB = 512  # q-block width
KB = 128  # k-chunk width
NQB = S // QB  # 4
NKB = S // KB  # 16

_CACHED = {}


def _build_nc():
    import concourse.bacc as bacc
    import concourse.mybir as mybir
    from concourse.tile import TileContext
    from concourse.masks import make_identity

    f32 = mybir.dt.float32
    f32r = mybir.dt.float32r
    EXP = mybir.ActivationFunctionType.Exp

    nc = bacc.Bacc()
    Qd = nc.declare_dram_parameter("Qt", [HPC, D, S], f32, isOutput=False)
    Kd = nc.declare_dram_parameter("Kt", [HPC, D, S], f32, isOutput=False)
    Vd = nc.declare_dram_parameter("V", [HPC, S, D], f32, isOutput=False)
    Od = nc.declare_dram_parameter("out", [HPC, S, D], f32, isOutput=True)

    with TileContext(nc) as tc:
        with (
            tc.tile_pool(name="consts", bufs=1) as cpool,
            tc.tile_pool(name="qt", bufs=2) as qt_pool,
            tc.tile_pool(name="kt", bufs=2) as kt_pool,
            tc.tile_pool(name="vp", bufs=2) as v_pool,
            tc.tile_pool(name="e", bufs=4) as e_pool,
            tc.tile_pool(name="ot", bufs=2) as ot_pool,
            tc.tile_pool(name="oo", bufs=2) as oo_pool,
            tc.tile_pool(name="r", bufs=2) as r_pool,
            tc.tile_pool(name="ps", bufs=3, space="PSUM") as ps_pool,
            tc.tile_pool(name="po", bufs=1, space="PSUM") as po_pool,
            tc.tile_pool(name="pt", bufs=1, space="PSUM") as pt_pool,
        ):
            # constants: identity for PE transpose, diagonal-block causal bias
            ident = cpool.tile([128, 128], f32)
            make_identity(nc, ident[:])
            # diag[i, j] = 0 if j >= i else -1e30   (valid when q_off >= k_off)
            diag = cpool.tile([128, 128], f32)
            nc.gpsimd.memset(diag[:], 0.0)
            nc.gpsimd.affine_select(
                out=diag[:],
                in_=diag[:],
                compare_op=mybir.AluOpType.is_ge,
                fill=-1e30,
                base=0,
                pattern=[[1, 128]],
                channel_multiplier=-1,
            )

            for h in range(HPC):
                # ---- load inputs for this head ----
                qt = qt_pool.tile([D, S], f32r, tag="qt")
                nc.sync.dma_start(out=qt[:], in_=Qd[h].bitcast(f32r))
                kt = kt_pool.tile([D, S], f32r, tag="kt")
                nc.sync.dma_start(out=kt[:], in_=Kd[h].bitcast(f32r))
                vp = v_pool.tile([128, NKB, 65], f32r, tag="vp")
                nc.sync.dma_start(
                    out=vp[:, :, 0:D],
                    in_=Vd[h].rearrange("(c p) d -> p c d", p=128).bitcast(f32r),
                )
                nc.gpsimd.memset(vp[:, :, D].bitcast(f32), 1.0)

                for qb in range(NQB):
                    q0 = qb * QB
                    nk = 4 * qb + 4  # causal k-chunks: 0..nk-1
                    po = po_pool.tile([D + 1, QB], f32, tag="po")

                    # units: pairs of full chunks, then 4 straddle chunks
                    units = [("pair", ki) for ki in range(0, 4 * qb, 2)]
                    units += [("str", ki) for ki in range(4 * qb, nk)]

                    pending = []  # PV emission lags one unit (keeps ACT fed)

                    def emit_pvs(pvs):
                        for ki, e_ap in pvs:
                            nc.tensor.matmul(
                                po[:],
                                lhsT=vp[:, ki, :],
                                rhs=e_ap,
                                start=(ki == 0),
                                stop=(ki == nk - 1),
                            )

                    for kind, ki in units:
                        ps = ps_pool.tile([KB, 2 * QB], f32, tag="ps")
                        e = e_pool.tile([KB, 2 * QB], f32r, tag="e")
                        if kind == "pair":
                            for half in (0, 1):
                                nc.tensor.matmul(
                                    ps[:, half * QB : (half + 1) * QB],
                                    lhsT=kt[:, (ki + half) * KB : (ki + half + 1) * KB],
                                    rhs=qt[:, q0 : q0 + QB],
                                    start=True,
                                    stop=True,
                                )
                            nc.scalar.activation(e[:], ps[:], EXP, scale=0.125)
                            pvs = [(ki, e[:, 0:QB]), (ki + 1, e[:, QB : 2 * QB])]
                        else:
                            off = ki * KB - q0  # 0, 128, 256, 384
                            nc.tensor.matmul(
                                ps[:, off:QB],
                                lhsT=kt[:, ki * KB : (ki + 1) * KB],
                                rhs=qt[:, q0 + off : q0 + QB],
                                start=True,
                                stop=True,
                            )
                            # additive -1e30 on the diagonal 128-block
                            nc.vector.tensor_add(
                                ps[:, off : off + KB], ps[:, off : off + KB], diag[:]
                            )
                            if off > 0:  # zero the fully-masked prefix of e
                                nc.gpsimd.memset(e[:, 0:off].bitcast(f32), 0.0)
                            nc.scalar.activation(
                                e[:, off:QB], ps[:, off:QB], EXP, scale=0.125
                            )
                            pvs = [(ki, e[:, 0:QB])]
                        if pending:
                            emit_pvs(pending.pop())
                        pending.append(pvs)
                    emit_pvs(pending.pop())

                    # ---- epilogue: transpose + normalize + store ----
                    ot = ot_pool.tile([D + 1, QB], f32, tag="ot")
                    nc.vector.tensor_copy(ot[:], po[:])
                    pt = pt_pool.tile([128, 4, D + 1], f32, tag="pt")
                    for j in range(4):
                        nc.tensor.transpose(
                            pt[:, j, :],
                            ot[:, j * 128 : (j + 1) * 128],
                            ident[: D + 1, : D + 1],
                        )
                    r = r_pool.tile([128, 4], f32, tag="r")
                    nc.vector.reciprocal(r[:], pt[:, :, D])
                    oo = oo_pool.tile([128, 4, D], f32, tag="oo")
                    nc.vector.tensor_mul(
                        oo[:], pt[:, :, 0:D], r[:].unsqueeze(2).broadcast_to([128, 4, D])
                    )
                    nc.sync.dma_start(
                        out=Od[h, q0 : q0 + QB, :].rearrange("(j p) d -> p j d", p=128),
                        in_=oo[:],
                    )
    nc.finalize()
    return nc


def _get_nc():
    if "nc" not in _CACHED:
        _CACHED["nc"] = _build_nc()
    return _CACHED["nc"]


def kernel(Q, K, V, mask=None, **_ignored):
    from concourse.bass_utils import run_bass_kernel_spmd

    nc = _get_nc()
    Qr = np.ascontiguousarray(
        np.asarray(Q, dtype=np.float32).reshape(B * H, S, D).transpose(0, 2, 1)
    )
    Kr = np.ascontiguousarray(
        np.asarray(K, dtype=np.float32).reshape(B * H, S, D).transpose(0, 2, 1)
    )
    Vr = np.ascontiguousarray(np.asarray(V, dtype=np.float32).reshape(B * H, S, D))
    in_maps = [
        {
            "Qt": Qr[i * HPC : (i + 1) * HPC],
            "Kt": Kr[i * HPC : (i + 1) * HPC],
            "V": Vr[i * HPC : (i + 1) * HPC],
        }
        for i in range(N_CORES)
    ]
    res = run_bass_kernel_spmd(nc, in_maps, core_ids=list(range(N_CORES)))
    out = np.concatenate([res.results[i]["out"] for i in range(N_CORES)], axis=0)
    return out.reshape(B, H, S, D).astype(np.float32)


# revision 10
# speedup vs baseline: 1.2789x; 1.0545x over previous
"""Causal attention kernel for Trainium2 (8 NeuronCores).

Problem: B=2, H=16, S=2048, D=64 causal attention with a softmax whose
global-max subtraction cancels mathematically (softmax is shift-invariant),
so an unshifted softmax is numerically equivalent in f32.

Sharding: the 32 (b,h) heads are split 4-per-core across 8 cores
(head-parallel, no communication). Q and K are pre-transposed on the host to
[head, D, S] during shard prep so the on-chip [d, s] layout (contraction dim
d on partitions) loads with contiguous DMA.

Per-core kernel (per head, scores computed in S^T = [k, q] layout):
  - QK: S^T[k_chunk, q_block] = matmul(lhsT=Kt chunk [64,128],
    rhs=Qt block [64,512]) in float32r (FP22 multiply, full PE speed).
  - exp(0.125 * S^T) on ScalarE straight out of PSUM. To amortize ScalarE's
    per-instruction overhead (the bottleneck engine):
      * fully-causal chunk pairs share one [128,1024] activation;
      * the three partial diagonal chunks of each q-block (offsets 128/256/
        384) write causally-trimmed score segments PACKED into one
        [128,768] PSUM tile and share ONE activation;
      * the offset-0 diagonal chunk is a full-width single.
    Diagonal 128-blocks get a -1e30 additive causal mask on VectorE before
    the exp.
  - PV: psum[65, 512] += matmul(lhsT=Vplus chunk [128,65], rhs=e) where
    Vplus has a ones column appended so row 64 accumulates softmax row-sums.
  - Epilogue: copy PV psum to SBUF, 4 PE-transposes into one [128, 4*65]
    PSUM tile, single reciprocal + broadcast multiply, one DMA per q-block.

Scheduling: input DMAs are chunked per q-window and split across the SP and
ACT HWDGE queues so compute starts early; PV matmuls lag their unit by two
(so ScalarE is always fed across block boundaries); each block's epilogue is
emitted right after its last PV pops from the lag queue; the final block's
epilogue is split per 128-row group to shorten the kernel tail.
"""

import numpy as np

B, H, S, D = 2, 16, 2048, 64
N_CORES = 8
HPC = (B * H) // N_CORES  # heads per core = 4
QB = 512  # q-block width
KB = 128  # k-chunk width
NQB = S // QB  # 4
NKB = S // KB  # 16

_CACHED = {}


def _build_nc():
    import concourse.bacc as bacc
    import concourse.mybir as mybir
    from concourse.tile import TileContext
    from concourse.masks import make_identity

    f32 = mybir.dt.float32
    f32r = mybir.dt.float32r
    EXP = mybir.ActivationFunctionType.Exp

    nc = bacc.Bacc()
    Qd = nc.declare_dram_parameter("Qt", [HPC, D, S], f32, isOutput=False)
    Kd = nc.declare_dram_parameter("Kt", [HPC, D, S], f32, isOutput=False)
    Vd = nc.declare_dram_parameter("V", [HPC, S, D], f32, isOutput=False)
    Od = nc.declare_dram_parameter("out", [HPC, S, D], f32, isOutput=True)

    with TileContext(nc) as tc:
        with (
            tc.tile_pool(name="consts", bufs=1) as cpool,
            tc.tile_pool(name="qt", bufs=2) as qt_pool,
            tc.tile_pool(name="kt", bufs=2) as kt_pool,
            tc.tile_pool(name="vp", bufs=2) as v_pool,
            tc.tile_pool(name="e", bufs=5) as e_pool,
            tc.tile_pool(name="ot", bufs=2) as ot_pool,
            tc.tile_pool(name="oo", bufs=2) as oo_pool,
            tc.tile_pool(name="r", bufs=2) as r_pool,
            tc.tile_pool(name="ps", bufs=2, space="PSUM") as ps_pool,
            tc.tile_pool(name="psk", bufs=1, space="PSUM") as psk_pool,
            tc.tile_pool(name="po", bufs=1, space="PSUM") as po_pool,
            tc.tile_pool(name="pt", bufs=1, space="PSUM") as pt_pool,
        ):
            # constants: identity for PE transpose, diagonal-block causal bias
            ident = cpool.tile([128, 128], f32)
            make_identity(nc, ident[:])
            # diag[i, j] = 0 if j >= i else -1e30   (valid when q_off >= k_off)
            diag = cpool.tile([128, 128], f32)
            nc.gpsimd.memset(diag[:], 0.0)
            nc.gpsimd.affine_select(
                out=diag[:],
                in_=diag[:],
                compare_op=mybir.AluOpType.is_ge,
                fill=-1e30,
                base=0,
                pattern=[[1, 128]],
                channel_multiplier=-1,
            )

            def load_head(h):
                qt = qt_pool.tile([D, S], f32r, tag="qt")
                kt = kt_pool.tile([D, S], f32r, tag="kt")
                vp = v_pool.tile([128, NKB, 65], f32r, tag="vp")
                nc.gpsimd.memset(vp[:, :, D].bitcast(f32), 1.0)
                vr = Vd[h].rearrange("(c p) d -> p c d", p=128).bitcast(f32r)
                for c in range(NQB):  # chunked so the first q-window starts early
                    sl = slice(c * QB, (c + 1) * QB)
                    nc.sync.dma_start(out=kt[:, sl], in_=Kd[h, :, sl].bitcast(f32r))
                    nc.scalar.dma_start(out=qt[:, sl], in_=Qd[h, :, sl].bitcast(f32r))
                    nc.sync.dma_start(
                        out=vp[:, 4 * c : 4 * c + 4, 0:D],
                        in_=vr[:, 4 * c : 4 * c + 4, :],
                    )
                return qt, kt, vp

            def emit_epilogue(h, qb, po, split):
                q0 = qb * QB
                ot = ot_pool.tile([D + 1, QB], f32, tag="ot")
                pt = pt_pool.tile([128, 4, D + 1], f32, tag="pt")
                if not split:
                    nc.vector.tensor_copy(ot[:], po[:])
                    for j in range(4):
                        nc.tensor.transpose(
                            pt[:, j, :],
                            ot[:, j * 128 : (j + 1) * 128],
                            ident[: D + 1, : D + 1],
                        )
                    r = r_pool.tile([128, 4], f32, tag="r")
                    nc.vector.reciprocal(r[:], pt[:, :, D])
                    oo = oo_pool.tile([128, 4, D], f32, tag="oo")
                    nc.vector.tensor_mul(
                        oo[:],
                        pt[:, :, 0:D],
                        r[:].unsqueeze(2).broadcast_to([128, 4, D]),
                    )
                    nc.sync.dma_start(
                        out=Od[h, q0 : q0 + QB, :].rearrange("(j p) d -> p j d", p=128),
                        in_=oo[:],
                    )
                else:  # fine-grained tail for the very last block
                    r = r_pool.tile([128, 4], f32, tag="r")
                    oo = oo_pool.tile([128, 4, D], f32, tag="oo")
                    for j in range(4):
                        jsl = slice(j * 128, (j + 1) * 128)
                        nc.vector.tensor_copy(ot[:, jsl], po[:, jsl])
                        nc.tensor.transpose(
                            pt[:, j, :], ot[:, jsl], ident[: D + 1, : D + 1]
                        )
                        nc.vector.reciprocal(r[:, j : j + 1], pt[:, j, D : D + 1])
                        nc.vector.tensor_mul(
                            oo[:, j, :],
                            pt[:, j, 0:D],
                            r[:, j : j + 1].broadcast_to([128, D]),
                        )
                        nc.sync.dma_start(
                            out=Od[h, q0 + j * 128 : q0 + (j + 1) * 128, :],
                            in_=oo[:, j, :],
                        )

            # global software pipeline: PV emission lags its unit by two
            pv_queue = []  # [(po, vp, nk, [(ki, e_ap, colrange)], epilogue|None)]

            def pop_pv():
                po_t, vp_t, nk_t, pvs, epi = pv_queue.pop(0)
                for ki, e_ap, cols in pvs:
                    nc.tensor.matmul(
                        po_t[:, cols],
                        lhsT=vp_t[:, ki, :],
                        rhs=e_ap,
                        start=(ki == 0),
                        stop=(ki == nk_t - 1),
                    )
                if epi is not None:
                    epi()

            for h in range(HPC):
                qt, kt, vp = load_head(h)
                for qb in range(NQB):
                    q0 = qb * QB
                    nk = 4 * qb + 4
                    po = po_pool.tile([D + 1, QB], f32, tag="po")

                    # units: pairs over the 4*qb full chunks + the off=0
                    # diagonal chunk (odd one out as a single), then the
                    # packed partial-diagonal unit (offs 128/256/384).
                    width_units = []
                    fulls = list(range(4 * qb + 1))  # ki of full-width chunks
                    for i in range(0, len(fulls) - 1, 2):
                        width_units.append(("pair", fulls[i]))
                    width_units.append(("single", fulls[-1]))
                    width_units.append(("packed", 4 * qb + 1))

                    for ui, (kind, ki) in enumerate(width_units):
                        is_last = kind == "packed"
                        if kind == "pair":
                            ps = ps_pool.tile([KB, 2 * QB], f32, tag="ps")
                            e = e_pool.tile([KB, 2 * QB], f32r, tag="e")
                            for half in (0, 1):
                                nc.tensor.matmul(
                                    ps[:, half * QB : (half + 1) * QB],
                                    lhsT=kt[
                                        :, (ki + half) * KB : (ki + half + 1) * KB
                                    ],
                                    rhs=qt[:, q0 : q0 + QB],
                                    start=True,
                                    stop=True,
                                )
                            nc.scalar.activation(e[:], ps[:], EXP, scale=0.125)
                            pvs = [
                                (ki, e[:, 0:QB], slice(0, QB)),
                                (ki + 1, e[:, QB : 2 * QB], slice(0, QB)),
                            ]
                        elif kind == "single":  # off=0 diagonal chunk, full width
                            ps = ps_pool.tile([KB, 2 * QB], f32, tag="ps")
                            e = e_pool.tile([KB, 2 * QB], f32r, tag="e")
                            nc.tensor.matmul(
                                ps[:, 0:QB],
                                lhsT=kt[:, ki * KB : (ki + 1) * KB],
                                rhs=qt[:, q0 : q0 + QB],
                                start=True,
                                stop=True,
                            )
                            nc.vector.tensor_add(ps[:, 0:KB], ps[:, 0:KB], diag[:])
                            nc.scalar.activation(
                                e[:, 0:QB], ps[:, 0:QB], EXP, scale=0.125
                            )
                            pvs = [(ki, e[:, 0:QB], slice(0, QB))]
                        else:  # packed partial-diagonal chunks, bank-aligned:
                            # off=128 -> [0:384], off=384 -> [384:512],
                            # off=256 -> [512:768]  (matmul outs must not
                            # cross a 512-f32 PSUM bank boundary)
                            ps = psk_pool.tile([KB, 768], f32, tag="psk")
                            e = e_pool.tile([KB, 2 * QB], f32r, tag="e")
                            segs = []  # (ki, off, base, width)
                            for off, base in ((KB, 0), (3 * KB, 384), (2 * KB, 512)):
                                w = QB - off
                                kk = 4 * qb + off // KB
                                nc.tensor.matmul(
                                    ps[:, base : base + w],
                                    lhsT=kt[:, kk * KB : (kk + 1) * KB],
                                    rhs=qt[:, q0 + off : q0 + QB],
                                    start=True,
                                    stop=True,
                                )
                                nc.vector.tensor_add(
                                    ps[:, base : base + KB],
                                    ps[:, base : base + KB],
                                    diag[:],
                                )
                                segs.append((kk, off, base, w))
                            nc.scalar.activation(
                                e[:, 0:768], ps[:], EXP, scale=0.125
                            )
                            pvs = [
                                (kk, e[:, base : base + w], slice(off, QB))
                                for kk, off, base, w in segs
                            ]

                        epi = None
                        if is_last:
                            last_block = h == HPC - 1 and qb == NQB - 1

                            def make_epi(h=h, qb=qb, po=po, split=last_block):
                                return lambda: emit_epilogue(h, qb, po, split)

                            epi = make_epi()
                        pv_queue.append((po, vp, nk, pvs, epi))
                        while len(pv_queue) > 2:
                            pop_pv()

            while pv_queue:
                pop_pv()
    nc.finalize()
    return nc


def _get_nc():
    if "nc" not in _CACHED:
        _CACHED["nc"] = _build_nc()
    return _CACHED["nc"]


def kernel(Q, K, V, mask=None, **_ignored):
    from concourse.bass_utils import run_bass_kernel_spmd

    nc = _get_nc()
    Qr = np.ascontiguousarray(
        np.asarray(Q, dtype=np.float32).reshape(B * H, S, D).transpose(0, 2, 1)
    )
    Kr = np.ascontiguousarray(
        np.asarray(K, dtype=np.float32).reshape(B * H, S, D).transpose(0, 2, 1)
    )
    Vr = np.ascontiguousarray(np.asarray(V, dtype=np.float32).reshape(B * H, S, D))
    in_maps = [
        {
            "Qt": Qr[i * HPC : (i + 1) * HPC],
            "Kt": Kr[i * HPC : (i + 1) * HPC],
            "V": Vr[i * HPC : (i + 1) * HPC],
        }
        for i in range(N_CORES)
    ]
    res = run_bass_kernel_spmd(nc, in_maps, core_ids=list(range(N_CORES)))
    out = np.concatenate([res.results[i]["out"] for i in range(N_CORES)], axis=0)
    return out.reshape(B, H, S, D).astype(np.float32)


# revision 17
# speedup vs baseline: 1.3951x; 1.0909x over previous
"""Causal attention kernel for Trainium2 (8 NeuronCores).

Problem: B=2, H=16, S=2048, D=64 causal attention with a softmax whose
global-max subtraction cancels mathematically (softmax is shift-invariant),
so an unshifted softmax is numerically equivalent in f32.

Sharding: the 32 (b,h) heads are split 4-per-core across 8 cores
(head-parallel, no communication). Q and K are pre-transposed on the host to
[head, D, S] during shard prep so the on-chip [d, s] layout (contraction dim
d on partitions) loads with contiguous DMA.

Per-core kernel (per head, scores computed in S^T = [k, q] layout):
  - QK: S^T[k_chunk, q_block] = matmul(lhsT=Kt chunk [64,128],
    rhs=Qt block [64,512]) in float32r (FP22 multiply, full PE speed).
  - exp(0.125 * S^T) on ScalarE straight out of PSUM. To amortize ScalarE's
    per-instruction overhead (the bottleneck engine):
      * fully-causal chunk pairs share one [128,1024] activation;
      * the three partial diagonal chunks of each q-block (offsets 128/256/
        384) write causally-trimmed score segments PACKED into one
        [128,768] PSUM tile and share ONE activation;
      * the offset-0 diagonal chunk is a full-width single.
    Diagonal 128-blocks get a -1e30 additive causal mask on VectorE before
    the exp.
  - PV: psum[65, 512] += matmul(lhsT=Vplus chunk [128,65], rhs=e) where
    Vplus has a ones column appended so row 64 accumulates softmax row-sums.
  - Epilogue: copy PV psum to SBUF, 4 PE-transposes into one [128, 4*65]
    PSUM tile, single reciprocal + broadcast multiply, one DMA per q-block.

Scheduling: input DMAs are chunked per q-window and split across the SP and
ACT HWDGE queues so compute starts early; PV matmuls lag their unit by two
(so ScalarE is always fed across block boundaries); each block's epilogue is
emitted right after its last PV pops from the lag queue; the final block's
epilogue is split per 128-row group to shorten the kernel tail.
"""

import numpy as np

B, H, S, D = 2, 16, 2048, 64
N_CORES = 8
HPC = (B * H) // N_CORES  # heads per core = 4
QB = 512  # q-block width
KB = 128  # k-chunk width
NQB = S // QB  # 4
NKB = S // KB  # 16

_CACHED = {}


def _build_nc():
    import concourse.bacc as bacc
    import concourse.mybir as mybir
    from concourse.tile import TileContext
    from concourse.masks import make_identity

    f32 = mybir.dt.float32
    f32r = mybir.dt.float32r
    EXP = mybir.ActivationFunctionType.Exp

    nc = bacc.Bacc()
    Qd = nc.declare_dram_parameter("Qt", [HPC, D, S], f32, isOutput=False)
    Kd = nc.declare_dram_parameter("Kt", [HPC, D, S], f32, isOutput=False)
    Vd = nc.declare_dram_parameter("V", [HPC, S, D], f32, isOutput=False)
    Od = nc.declare_dram_parameter("out", [HPC, S, D], f32, isOutput=True)

    with TileContext(nc) as tc:
        with (
            tc.tile_pool(name="consts", bufs=1) as cpool,
            tc.tile_pool(name="qt", bufs=3) as qt_pool,
            tc.tile_pool(name="kt", bufs=3) as kt_pool,
            tc.tile_pool(name="vp", bufs=3) as v_pool,
            tc.tile_pool(name="e", bufs=5) as e_pool,
            tc.tile_pool(name="ot", bufs=3) as ot_pool,
            tc.tile_pool(name="oo", bufs=3) as oo_pool,
            tc.tile_pool(name="r", bufs=2) as r_pool,
            tc.tile_pool(name="ps", bufs=2, space="PSUM") as ps_pool,
            tc.tile_pool(name="psk", bufs=1, space="PSUM") as psk_pool,
            tc.tile_pool(name="po", bufs=1, space="PSUM") as po_pool,
            tc.tile_pool(name="pt", bufs=1, space="PSUM") as pt_pool,
        ):
            # constants: identity for PE transpose, diagonal-block causal bias
            ident = cpool.tile([128, 128], f32)
            make_identity(nc, ident[:])

            def causal_zero(e_blk):
                # zero e[i, j] for j < i (future positions) on the idle
                # GpSimd engine — keeps masking off the ACT critical path
                nc.gpsimd.affine_select(
                    out=e_blk,
                    in_=e_blk,
                    compare_op=mybir.AluOpType.is_ge,
                    fill=0.0,
                    base=0,
                    pattern=[[1, KB]],
                    channel_multiplier=-1,
                )

            def load_head(h):
                qt = qt_pool.tile([D, S], f32r, tag="qt")
                kt = kt_pool.tile([D, S], f32r, tag="kt")
                vp = v_pool.tile([128, NKB, 65], f32r, tag="vp")
                nc.gpsimd.memset(vp[:, :, D].bitcast(f32), 1.0)
                vr = Vd[h].rearrange("(c p) d -> p c d", p=128).bitcast(f32r)
                # DMA issue costs ~0.6us serialized per queue: minimize DMA
                # count.  Head 0 is latency-critical (nothing else to overlap
                # with) so it splits each tensor in two; later heads load
                # whole tensors, prefetched behind the previous head.
                if h == 0:
                    # prologue: nothing to overlap with — use the fast HWDGE
                    # queues, smallest chunks first so qb0 starts ASAP
                    parts = [(0, 512), (512, 1024), (1024, 2048)]
                    for ci, (a, b) in enumerate(parts):
                        sl = slice(a, b)
                        nc.sync.dma_start(
                            out=kt[:, sl], in_=Kd[h, :, sl].bitcast(f32r)
                        )
                        nc.scalar.dma_start(
                            out=qt[:, sl], in_=Qd[h, :, sl].bitcast(f32r)
                        )
                        csl = slice(a // KB, b // KB)
                        nc.sync.dma_start(out=vp[:, csl, 0:D], in_=vr[:, csl, :])
                else:
                    # steady state: SWDGE on the idle GpSimd engine, keeping
                    # the HWDGE queues free for output stores and off the
                    # ACT/SP instruction streams
                    nc.gpsimd.dma_start(out=kt[:], in_=Kd[h].bitcast(f32r))
                    nc.gpsimd.dma_start(out=qt[:], in_=Qd[h].bitcast(f32r))
                    nc.gpsimd.dma_start(out=vp[:, :, 0:D], in_=vr[:])
                return qt, kt, vp

            def epilogue_a(po):
                # frees the po PSUM accumulator ASAP (po pool has one buffer)
                ot = ot_pool.tile([D + 1, QB], f32, tag="ot")
                nc.vector.tensor_copy(ot[:], po[:])
                return ot

            def epilogue_b(h, qb, ot):
                q0 = qb * QB
                pt = pt_pool.tile([128, 4, D + 1], f32, tag="pt")
                for j in range(4):
                    nc.tensor.transpose(
                        pt[:, j, :],
                        ot[:, j * 128 : (j + 1) * 128],
                        ident[: D + 1, : D + 1],
                    )
                r = r_pool.tile([128, 4], f32, tag="r")
                nc.vector.reciprocal(r[:], pt[:, :, D])
                oo = oo_pool.tile([128, 4, D], f32, tag="oo")
                nc.vector.tensor_mul(
                    oo[:],
                    pt[:, :, 0:D],
                    r[:].unsqueeze(2).broadcast_to([128, 4, D]),
                )
                nc.sync.dma_start(
                    out=Od[h, q0 : q0 + QB, :].rearrange("(j p) d -> p j d", p=128),
                    in_=oo[:],
                )

            # global software pipeline: a FIFO of deferred actions (PV
            # matmuls and epilogue halves); up to two actions pop after each
            # emitted unit, so PVs/epilogues trail the QK/exp stream without
            # ever clumping at block boundaries.
            actions = []

            def pump(limit=2):
                n = 0
                while actions and len(actions) > 2 and n < limit:
                    actions.pop(0)()
                    n += 1

            def make_pv(po, vp, nk, pvs):
                def act():
                    for ki, e_ap, cols in pvs:
                        nc.tensor.matmul(
                            po[:, cols],
                            lhsT=vp[:, ki, :],
                            rhs=e_ap,
                            start=(ki == 0),
                            stop=(ki == nk - 1),
                        )

                return act

            for h in range(HPC):
                qt, kt, vp = load_head(h)
                for qb in range(NQB):
                    q0 = qb * QB
                    nk = 4 * qb + 4
                    po = po_pool.tile([D + 1, QB], f32, tag="po")

                    # units: pairs over the 4*qb full chunks + the off=0
                    # diagonal chunk (odd one out as a single), then the
                    # packed partial-diagonal unit (offs 128/256/384).
                    fulls = list(range(4 * qb + 1))
                    units = [("pair", fulls[i]) for i in range(0, len(fulls) - 1, 2)]
                    units.append(("single", fulls[-1]))
                    units.append(("packed", 0))

                    for kind, ki in units:
                        e = e_pool.tile([KB, 2 * QB], f32r, tag="e")
                        if kind == "pair":
                            ps = ps_pool.tile([KB, 2 * QB], f32, tag="ps")
                            for half in (0, 1):
                                nc.tensor.matmul(
                                    ps[:, half * QB : (half + 1) * QB],
                                    lhsT=kt[
                                        :, (ki + half) * KB : (ki + half + 1) * KB
                                    ],
                                    rhs=qt[:, q0 : q0 + QB],
                                    start=True,
                                    stop=True,
                                )
                            nc.scalar.activation(e[:], ps[:], EXP, scale=0.125)
                            pvs = [
                                (ki, e[:, 0:QB], slice(0, QB)),
                                (ki + 1, e[:, QB : 2 * QB], slice(0, QB)),
                            ]
                        elif kind == "single":  # off=0 diagonal chunk, full width
                            ps = ps_pool.tile([KB, 2 * QB], f32, tag="ps")
                            nc.tensor.matmul(
                                ps[:, 0:QB],
                                lhsT=kt[:, ki * KB : (ki + 1) * KB],
                                rhs=qt[:, q0 : q0 + QB],
                                start=True,
                                stop=True,
                            )
                            nc.scalar.activation(
                                e[:, 0:QB], ps[:, 0:QB], EXP, scale=0.125
                            )
                            causal_zero(e[:, 0:KB])
                            pvs = [(ki, e[:, 0:QB], slice(0, QB))]
                        else:  # packed partial-diagonal chunks, bank-aligned:
                            # off=128 -> [0:384], off=384 -> [384:512],
                            # off=256 -> [512:768]  (matmul outs must not
                            # cross a 512-f32 PSUM bank boundary)
                            ps = psk_pool.tile([KB, 768], f32, tag="psk")
                            segs = []
                            for off, base in ((KB, 0), (3 * KB, 384), (2 * KB, 512)):
                                w = QB - off
                                kk = 4 * qb + off // KB
                                nc.tensor.matmul(
                                    ps[:, base : base + w],
                                    lhsT=kt[:, kk * KB : (kk + 1) * KB],
                                    rhs=qt[:, q0 + off : q0 + QB],
                                    start=True,
                                    stop=True,
                                )
                                segs.append((kk, off, base, w))
                            nc.scalar.activation(e[:, 0:768], ps[:], EXP, scale=0.125)
                            for kk, off, base, w in segs:
                                causal_zero(e[:, base : base + KB])
                            pvs = [
                                (kk, e[:, base : base + w], slice(off, QB))
                                for kk, off, base, w in segs
                            ]

                        actions.append(make_pv(po, vp, nk, pvs))
                        pump()

                    def make_epis(h=h, qb=qb, po=po):
                        box = {}

                        def act_a():
                            box["ot"] = epilogue_a(po)

                        def act_b():
                            epilogue_b(h, qb, box["ot"])

                        return act_a, act_b

                    a, b = make_epis()
                    actions.append(a)
                    actions.append(b)

            while actions:
                actions.pop(0)()
    nc.finalize()
    return nc


def _get_nc():
    if "nc" not in _CACHED:
        _CACHED["nc"] = _build_nc()
    return _CACHED["nc"]


def kernel(Q, K, V, mask=None, **_ignored):
    from concourse.bass_utils import run_bass_kernel_spmd

    nc = _get_nc()
    Qr = np.ascontiguousarray(
        np.asarray(Q, dtype=np.float32).reshape(B * H, S, D).transpose(0, 2, 1)
    )
    Kr = np.ascontiguousarray(
        np.asarray(K, dtype=np.float32).reshape(B * H, S, D).transpose(0, 2, 1)
    )
    Vr = np.ascontiguousarray(np.asarray(V, dtype=np.float32).reshape(B * H, S, D))
    in_maps = [
        {
            "Qt": Qr[i * HPC : (i + 1) * HPC],
            "Kt": Kr[i * HPC : (i + 1) * HPC],
            "V": Vr[i * HPC : (i + 1) * HPC],
        }
        for i in range(N_CORES)
    ]
    res = run_bass_kernel_spmd(nc, in_maps, core_ids=list(range(N_CORES)))
    out = np.concatenate([res.results[i]["out"] for i in range(N_CORES)], axis=0)
    return out.reshape(B, H, S, D).astype(np.float32)


# revision 23
# speedup vs baseline: 1.3981x; 1.0021x over previous
"""Causal attention kernel for Trainium2 (8 NeuronCores).

Problem: B=2, H=16, S=2048, D=64 causal attention with a softmax whose
global-max subtraction cancels mathematically (softmax is shift-invariant),
so an unshifted softmax is numerically equivalent in f32.

Sharding: the 32 (b,h) heads are split 4-per-core across 8 cores
(head-parallel, no communication). Q and K are pre-transposed on the host to
[head, D, S] during shard prep so the on-chip [d, s] layout (contraction dim
d on partitions) loads with contiguous DMA.

Per-core kernel (per head, scores computed in S^T = [k, q] layout):
  - QK: S^T[k_chunk, q_block] = matmul(lhsT=Kt chunk [64,128],
    rhs=Qt block [64,512]) in float32r (FP22 multiply, full PE speed).
  - exp(0.125 * S^T) on ScalarE straight out of PSUM. To amortize ScalarE's
    per-instruction overhead (the bottleneck engine):
      * fully-causal chunk pairs share one [128,1024] activation;
      * the three partial diagonal chunks of each q-block (offsets 128/256/
        384) write causally-trimmed score segments PACKED into one
        [128,768] PSUM tile and share ONE activation;
      * the offset-0 diagonal chunk is a full-width single.
    Diagonal 128-blocks get a -1e30 additive causal mask on VectorE before
    the exp.
  - PV: psum[65, 512] += matmul(lhsT=Vplus chunk [128,65], rhs=e) where
    Vplus has a ones column appended so row 64 accumulates softmax row-sums.
  - Epilogue: copy PV psum to SBUF, 4 PE-transposes into one [128, 4*65]
    PSUM tile, single reciprocal + broadcast multiply, one DMA per q-block.

Scheduling: input DMAs are chunked per q-window and split across the SP and
ACT HWDGE queues so compute starts early; PV matmuls lag their unit by two
(so ScalarE is always fed across block boundaries); each block's epilogue is
emitted right after its last PV pops from the lag queue; the final block's
epilogue is split per 128-row group to shorten the kernel tail.
"""

import numpy as np

B, H, S, D = 2, 16, 2048, 64
N_CORES = 8
HPC = (B * H) // N_CORES  # heads per core = 4
QB = 512  # q-block width
KB = 128  # k-chunk width
NQB = S // QB  # 4
NKB = S // KB  # 16

_CACHED = {}


def _build_nc():
    import concourse.bacc as bacc
    import concourse.mybir as mybir
    from concourse.tile import TileContext
    from concourse.masks import make_identity

    f32 = mybir.dt.float32
    f32r = mybir.dt.float32r
    EXP = mybir.ActivationFunctionType.Exp

    nc = bacc.Bacc()
    Qd = nc.declare_dram_parameter("Qt", [HPC, D, S], f32, isOutput=False)
    Kd = nc.declare_dram_parameter("Kt", [HPC, D, S], f32, isOutput=False)
    Vd = nc.declare_dram_parameter("V", [HPC, S, D], f32, isOutput=False)
    Od = nc.declare_dram_parameter("out", [HPC, S, D], f32, isOutput=True)

    with TileContext(nc) as tc:
        with (
            tc.tile_pool(name="consts", bufs=1) as cpool,
            tc.tile_pool(name="qt", bufs=3) as qt_pool,
            tc.tile_pool(name="kt", bufs=3) as kt_pool,
            tc.tile_pool(name="vp", bufs=3) as v_pool,
            tc.tile_pool(name="e", bufs=5) as e_pool,
            tc.tile_pool(name="ot", bufs=3) as ot_pool,
            tc.tile_pool(name="oo", bufs=3) as oo_pool,
            tc.tile_pool(name="r", bufs=2) as r_pool,
            tc.tile_pool(name="ps", bufs=2, space="PSUM") as ps_pool,
            tc.tile_pool(name="psk", bufs=1, space="PSUM") as psk_pool,
            tc.tile_pool(name="po", bufs=1, space="PSUM") as po_pool,
            tc.tile_pool(name="pt", bufs=1, space="PSUM") as pt_pool,
        ):
            # PE warmup: dummy matmuls so the clock ramp starts at t=0, not
            # at the first real QK (outputs never read)
            bf16 = mybir.dt.bfloat16
            wa = cpool.tile([64, 128], bf16)
            wb = cpool.tile([64, 512], bf16)
            nc.vector.memset(wa[:], 0.0)
            nc.vector.memset(wb[:], 0.0)
            wp = ps_pool.tile([128, QB], f32, tag="ps")
            for _ in range(6):
                nc.tensor.matmul(wp[:, 0:QB], lhsT=wa[:], rhs=wb[:], start=True, stop=True)

            # constants: identity for PE transpose, diagonal-block causal bias
            ident = cpool.tile([128, 128], f32)
            make_identity(nc, ident[:])

            def causal_zero(e_blk):
                # zero e[i, j] for j < i (future positions) on the idle
                # GpSimd engine — keeps masking off the ACT critical path
                nc.gpsimd.affine_select(
                    out=e_blk,
                    in_=e_blk,
                    compare_op=mybir.AluOpType.is_ge,
                    fill=0.0,
                    base=0,
                    pattern=[[1, KB]],
                    channel_multiplier=-1,
                )

            def load_head(h):
                qt = qt_pool.tile([D, S], f32r, tag="qt")
                kt = kt_pool.tile([D, S], f32r, tag="kt")
                vp = v_pool.tile([128, NKB, 65], f32r, tag="vp")
                nc.gpsimd.memset(vp[:, :, D].bitcast(f32), 1.0)
                vr = Vd[h].rearrange("(c p) d -> p c d", p=128).bitcast(f32r)
                # DMA issue costs ~0.6us serialized per queue: minimize DMA
                # count.  Head 0 is latency-critical (nothing else to overlap
                # with) so it splits each tensor in two; later heads load
                # whole tensors, prefetched behind the previous head.
                if h == 0:
                    # prologue: nothing to overlap with — use the fast HWDGE
                    # queues, smallest chunks first so qb0 starts ASAP
                    parts = [(0, 512), (512, 1024), (1024, 2048)]
                    for ci, (a, b) in enumerate(parts):
                        sl = slice(a, b)
                        nc.sync.dma_start(
                            out=kt[:, sl], in_=Kd[h, :, sl].bitcast(f32r)
                        )
                        nc.scalar.dma_start(
                            out=qt[:, sl], in_=Qd[h, :, sl].bitcast(f32r)
                        )
                        csl = slice(a // KB, b // KB)
                        nc.sync.dma_start(out=vp[:, csl, 0:D], in_=vr[:, csl, :])
                else:
                    # steady state: SWDGE on the idle GpSimd engine, keeping
                    # the HWDGE queues free for output stores and off the
                    # ACT/SP instruction streams
                    nc.gpsimd.dma_start(out=kt[:], in_=Kd[h].bitcast(f32r))
                    nc.gpsimd.dma_start(out=qt[:], in_=Qd[h].bitcast(f32r))
                    nc.gpsimd.dma_start(out=vp[:, :, 0:D], in_=vr[:])
                return qt, kt, vp

            def epilogue_a(po):
                # frees the po PSUM accumulator ASAP (po pool has one buffer)
                ot = ot_pool.tile([D + 1, QB], f32, tag="ot")
                nc.vector.tensor_copy(ot[:], po[:])
                return ot

            def epilogue_last(h, qb, po):
                # fully per-j pipelined tail for the very last block
                q0 = qb * QB
                ot = ot_pool.tile([D + 1, QB], f32, tag="ot")
                pt = pt_pool.tile([128, 4, D + 1], f32, tag="pt")
                r = r_pool.tile([128, 4], f32, tag="r")
                oo = oo_pool.tile([128, 4, D], f32, tag="oo")
                for j in range(4):
                    jsl = slice(j * 128, (j + 1) * 128)
                    nc.vector.tensor_copy(ot[:, jsl], po[:, jsl])
                    nc.tensor.transpose(pt[:, j, :], ot[:, jsl], ident[: D + 1, : D + 1])
                    nc.vector.reciprocal(r[:, j : j + 1], pt[:, j, D : D + 1])
                    nc.vector.tensor_mul(
                        oo[:, j, :],
                        pt[:, j, 0:D],
                        r[:, j : j + 1].broadcast_to([128, D]),
                    )
                    nc.sync.dma_start(
                        out=Od[h, q0 + j * 128 : q0 + (j + 1) * 128, :],
                        in_=oo[:, j, :],
                    )

            def epilogue_b(h, qb, ot):
                q0 = qb * QB
                pt = pt_pool.tile([128, 4, D + 1], f32, tag="pt")
                for j in range(4):
                    nc.tensor.transpose(
                        pt[:, j, :],
                        ot[:, j * 128 : (j + 1) * 128],
                        ident[: D + 1, : D + 1],
                    )
                r = r_pool.tile([128, 4], f32, tag="r")
                nc.vector.reciprocal(r[:], pt[:, :, D])
                oo = oo_pool.tile([128, 4, D], f32, tag="oo")
                nc.vector.tensor_mul(
                    oo[:],
                    pt[:, :, 0:D],
                    r[:].unsqueeze(2).broadcast_to([128, 4, D]),
                )
                nc.sync.dma_start(
                    out=Od[h, q0 : q0 + QB, :].rearrange("(j p) d -> p j d", p=128),
                    in_=oo[:],
                )

            # global software pipeline: a FIFO of deferred actions (PV
            # matmuls and epilogue halves); up to two actions pop after each
            # emitted unit, so PVs/epilogues trail the QK/exp stream without
            # ever clumping at block boundaries.
            actions = []

            def pump(limit=2):
                n = 0
                while actions and len(actions) > 2 and n < limit:
                    actions.pop(0)()
                    n += 1

            def make_pv(po, vp, stop_ki, pvs):
                def act():
                    for ki, e_ap, cols in pvs:
                        nc.tensor.matmul(
                            po[:, cols],
                            lhsT=vp[:, ki, :],
                            rhs=e_ap,
                            start=(ki == 0),
                            stop=(ki == stop_ki),
                        )

                return act

            for h in range(HPC):
                qt, kt, vp = load_head(h)
                for qb in range(NQB):
                    q0 = qb * QB
                    nk = 4 * qb + 4
                    po = po_pool.tile([D + 1, QB], f32, tag="po")
                    last_pvs = []

                    # units: pairs over the 4*qb full chunks + the off=0
                    # diagonal chunk (odd one out as a single), then the
                    # packed partial-diagonal unit (offs 128/256/384).
                    fulls = list(range(4 * qb + 1))
                    units = [("pair", fulls[i]) for i in range(0, len(fulls) - 1, 2)]
                    last_block = False and (h == HPC - 1 and qb == NQB - 1)
                    if last_block:
                        units.append(("packed", 0))
                        units.append(("single", fulls[-1]))
                    else:
                        units.append(("single", fulls[-1]))
                        units.append(("packed", 0))
                    stop_ki = nk - 1

                    for kind, ki in units:
                        e = e_pool.tile([KB, 2 * QB], f32r, tag="e")
                        if kind == "pair":
                            ps = ps_pool.tile([KB, 2 * QB], f32, tag="ps")
                            for half in (0, 1):
                                nc.tensor.matmul(
                                    ps[:, half * QB : (half + 1) * QB],
                                    lhsT=kt[
                                        :, (ki + half) * KB : (ki + half + 1) * KB
                                    ],
                                    rhs=qt[:, q0 : q0 + QB],
                                    start=True,
                                    stop=True,
                                )
                            nc.scalar.activation(e[:], ps[:], EXP, scale=0.125)
                            pvs = [
                                (ki, e[:, 0:QB], slice(0, QB)),
                                (ki + 1, e[:, QB : 2 * QB], slice(0, QB)),
                            ]
                        elif kind == "single":  # off=0 diagonal chunk, full width
                            ps = ps_pool.tile([KB, 2 * QB], f32, tag="ps")
                            nc.tensor.matmul(
                                ps[:, 0:QB],
                                lhsT=kt[:, ki * KB : (ki + 1) * KB],
                                rhs=qt[:, q0 : q0 + QB],
                                start=True,
                                stop=True,
                            )
                            nc.scalar.activation(
                                e[:, 0:QB], ps[:, 0:QB], EXP, scale=0.125
                            )
                            causal_zero(e[:, 0:KB])
                            pvs = [(ki, e[:, 0:QB], slice(0, QB))]
                        else:  # packed partial-diagonal chunks, bank-aligned:
                            # off=128 -> [0:384], off=384 -> [384:512],
                            # off=256 -> [512:768]  (matmul outs must not
                            # cross a 512-f32 PSUM bank boundary)
                            ps = psk_pool.tile([KB, 768], f32, tag="psk")
                            segs = []
                            for off, base in ((KB, 0), (2 * KB, 512), (3 * KB, 384)):
                                w = QB - off
                                kk = 4 * qb + off // KB
                                nc.tensor.matmul(
                                    ps[:, base : base + w],
                                    lhsT=kt[:, kk * KB : (kk + 1) * KB],
                                    rhs=qt[:, q0 + off : q0 + QB],
                                    start=True,
                                    stop=True,
                                )
                                segs.append((kk, off, base, w))
                            nc.scalar.activation(e[:, 0:768], ps[:], EXP, scale=0.125)
                            for kk, off, base, w in segs:
                                causal_zero(e[:, base : base + KB])
                            pvs = [
                                (kk, e[:, base : base + w], slice(off, QB))
                                for kk, off, base, w in segs
                            ]

                        if last_block:
                            last_pvs.append(pvs)
                        else:
                            actions.append(make_pv(po, vp, stop_ki, pvs))
                        pump()

                    if last_block:
                        while actions:
                            actions.pop(0)()
                        # inline tail: each 128-column group of po completes
                        # at a known PV; run its epilogue chain immediately
                        # so the tail overlaps the remaining activations
                        ot = ot_pool.tile([D + 1, QB], f32, tag="ot")
                        pt = pt_pool.tile([128, 4, D + 1], f32, tag="pt")
                        r = r_pool.tile([128, 4], f32, tag="r")
                        oo = oo_pool.tile([128, 4, D], f32, tag="oo")

                        def epi_j(j):
                            jsl = slice(j * 128, (j + 1) * 128)
                            nc.vector.tensor_copy(ot[:, jsl], po[:, jsl])
                            nc.tensor.transpose(
                                pt[:, j, :], ot[:, jsl], ident[: D + 1, : D + 1]
                            )

                        flat = [pv for pvs in last_pvs for pv in pvs]
                        # single (ki == 4qb) executes last: its exp is the
                        # smallest, and columns [0:128] are the only ones
                        # still open by then
                        flat.sort(key=lambda t: (t[0] == 4 * qb, 0))
                        for ki, e_ap, cols in flat:
                            nc.tensor.matmul(
                                po[:, cols],
                                lhsT=vp[:, ki, :],
                                rhs=e_ap,
                                start=(ki == 0),
                                stop=(ki == 4 * qb),
                            )
                            if ki == 4 * qb:  # single last: cols [0:128] final
                                epi_j(0)
                            elif ki == 4 * qb + 1:
                                epi_j(1)
                            elif ki == 4 * qb + 2:
                                epi_j(2)
                            elif ki == 4 * qb + 3:
                                epi_j(3)
                        nc.vector.reciprocal(r[:], pt[:, :, D])
                        nc.vector.tensor_mul(
                            oo[:],
                            pt[:, :, 0:D],
                            r[:].unsqueeze(2).broadcast_to([128, 4, D]),
                        )
                        nc.sync.dma_start(
                            out=Od[h, q0 : q0 + QB, :].rearrange(
                                "(j p) d -> p j d", p=128
                            ),
                            in_=oo[:],
                        )
                    else:

                        def make_epis(h=h, qb=qb, po=po):
                            box = {}

                            def act_a():
                                box["ot"] = epilogue_a(po)

                            def act_b():
                                epilogue_b(h, qb, box["ot"])

                            return act_a, act_b

                        a, b = make_epis()
                        actions.append(a)
                        actions.append(b)

            while actions:
                actions.pop(0)()
    nc.finalize()
    return nc


def _get_nc():
    if "nc" not in _CACHED:
        _CACHED["nc"] = _build_nc()
    return _CACHED["nc"]


def kernel(Q, K, V, mask=None, **_ignored):
    from concourse.bass_utils import run_bass_kernel_spmd

    nc = _get_nc()
    Qr = np.ascontiguousarray(
        np.asarray(Q, dtype=np.float32).reshape(B * H, S, D).transpose(0, 2, 1)
    )
    Kr = np.ascontiguousarray(
        np.asarray(K, dtype=np.float32).reshape(B * H, S, D).transpose(0, 2, 1)
    )
    Vr = np.ascontiguousarray(np.asarray(V, dtype=np.float32).reshape(B * H, S, D))
    in_maps = [
        {
            "Qt": Qr[i * HPC : (i + 1) * HPC],
            "Kt": Kr[i * HPC : (i + 1) * HPC],
            "V": Vr[i * HPC : (i + 1) * HPC],
        }
        for i in range(N_CORES)
    ]
    res = run_bass_kernel_spmd(nc, in_maps, core_ids=list(range(N_CORES)))
    out = np.concatenate([res.results[i]["out"] for i in range(N_CORES)], axis=0)
    return out.reshape(B, H, S, D).astype(np.float32)
